# revision 22
# baseline (speedup 1.0000x reference)
"""Chamfer loss (ChamferDistanceL1-style) Trainium2 Bass kernel.

Problem: B=4 samples, N=M=4096 points, 3D. loss = mean_b 0.5*(m1_b + m2_b)
  m1 = masked mean over valid pred points of sqrt(min_m d[n,m])
  m2 = mean over target points of sqrt(min over *valid* n of d[n,m])
  d[n,m] = max(|p_n|^2 + |t_m|^2 - 2 p.t, 0)

Strategy (8 NeuronCores):
  - Host compacts each sample's pred points to the valid (label==1) subset
    (~halves the work), splits them across 2 cores -> 8 cores = 4 samples x 2.
  - Distances are produced by a single K=5 fp32 matmul per tile:
      lhsT col n = [-2px, -2py, -2pz, 1, |p_n|^2 (+BIG if padding)]
      rhs  col m = [ tx,   ty,   tz,  |t_m|^2, 1]
    so PSUM holds d[n,m] directly (before the max(.,0) clamp).
  - Per PSUM chunk [128, 2048]:
      ACT: negated fp16 copy PSUM -> SBUF (sole PSUM consumer, so the fp32
           matmuls never stall on PSUM slots)
      DVE: row max(-d) via two fp16 TT-max tree levels (2x DVE mode) plus a
           final 1x tensor_reduce; fp16 TT-max into the negated column
           accumulator (also 2x mode)
  - GPSIMD finishes each chunk's 128-way partition max (overlapped with the
    next chunk's compute); host does the final clamp/sqrt/means (tiny).
  - fp16 is a value rounding of already-exact fp32 distances (max-combining
    is exact in fp16): measured loss error ~1e-6 relative.
"""

import numpy as np

import concourse.bacc as bacc
import concourse.bass_isa as bass_isa
import concourse.tile as tile
from concourse import mybir
from concourse.bass_utils import run_bass_kernel_spmd

F32 = mybir.dt.float32
F16 = mybir.dt.float16
BIG = np.float32(1e10)  # matches the reference's masking constant
_NC_CACHE = {}

_P = 128          # partitions / rows per weight tile
_MM_FREE = 512    # fp32 matmul moving-dim limit (one PSUM bank)
_CHUNK = 2048     # PSUM chunk (4 banks); 2 bufs = all 8 banks


def _build_nc(r_tiles: int, m_pad: int):
    """Build + finalize the per-core Bass program for R=128*r_tiles pred rows
    and m_pad (multiple of _CHUNK) target columns."""
    R = r_tiles * _P
    n_chunks = m_pad // _CHUNK

    nc = bacc.Bacc("TRN2", target_bir_lowering=False)
    inp = nc.dram_tensor("inp", [5, R + m_pad], F32, kind="ExternalInput")
    rowmin_d = nc.dram_tensor("rowmin", [_P, r_tiles], F32, kind="ExternalOutput")
    colmax_d = nc.dram_tensor("colmax", [1, m_pad], F32, kind="ExternalOutput")
    warm_d = nc.dram_tensor("warm", [_P, 1], F32, kind="ExternalOutput")

    with tile.TileContext(nc) as tc:
        with tc.tile_pool(name="io", bufs=1) as io, \
             tc.tile_pool(name="ps", bufs=2, space="PSUM") as psp:
            # PE warmup: a dummy matmul during the input DMA starts the HAM
            # clock-gate ramp so real matmuls run closer to full clock.
            wsrc = io.tile([5, _MM_FREE], F32)
            nc.vector.memset(wsrc[:], 0.0)
            wps = psp.tile([_P, _MM_FREE], F32, tag="ps")
            nc.tensor.matmul(wps[:], wsrc[:, 0:_P], wsrc[:],
                             start=True, stop=True)
            warm_sb = io.tile([_P, 1], F32)
            nc.vector.tensor_reduce(warm_sb[:], wps[:],
                                    axis=mybir.AxisListType.X,
                                    op=mybir.AluOpType.max)
            nc.sync.dma_start(out=warm_d[:, :], in_=warm_sb[:])

            # weights first (small), then one DMA per rhs chunk: the first
            # matmuls only wait on their own chunk's DMA.
            in_sb = io.tile([5, R + m_pad], F32)
            nc.sync.dma_start(out=in_sb[:, :R], in_=inp[:, :R])
            for c in range(n_chunks):
                cs = slice(R + c * _CHUNK, R + (c + 1) * _CHUNK)
                nc.sync.dma_start(out=in_sb[:, cs], in_=inp[:, cs])

            # negated fp16 column accumulator: holds max(-d) = -min(d)
            colacc = io.tile([_P, m_pad], F16)
            nc.any.memset(colacc[:], -60000.0)
            colred = io.tile([_P, m_pad], F32)

            rowstage = io.tile([_P, r_tiles * n_chunks], F32)

            with tc.tile_pool(name="scr", bufs=3) as scrp:
                for c in range(n_chunks):
                    for i in range(r_tiles):
                        lhsT = in_sb[:, i * _P:(i + 1) * _P]
                        ps = psp.tile([_P, _CHUNK], F32, tag="ps")
                        for s in range(_CHUNK // _MM_FREE):
                            col0 = R + c * _CHUNK + s * _MM_FREE
                            nc.tensor.matmul(
                                ps[:, s * _MM_FREE:(s + 1) * _MM_FREE],
                                lhsT,
                                in_sb[:, col0:col0 + _MM_FREE],
                                start=True, stop=True,
                            )
                        # ACT: scr = -d in fp16; frees the PSUM slot fast so
                        # the PE never stalls. Both reductions read scr.
                        scr = scrp.tile([_P, _CHUNK], F16, tag="scr")
                        nc.scalar.mul(scr[:], ps[:], -1.0)
                        # row max(-d): two fp16 TT-max tree levels run in the
                        # DVE 2x mode before the (1x-only) tensor_reduce.
                        h1 = _CHUNK // 2
                        s1 = scrp.tile([_P, h1], F16, tag="s1")
                        nc.vector.tensor_tensor(out=s1[:], in0=scr[:, :h1],
                                                in1=scr[:, h1:],
                                                op=mybir.AluOpType.max)
                        h2 = h1 // 2
                        s2 = scrp.tile([_P, h2], F16, tag="s2")
                        nc.vector.tensor_tensor(out=s2[:], in0=s1[:, :h2],
                                                in1=s1[:, h2:],
                                                op=mybir.AluOpType.max)
                        k = i * n_chunks + c
                        nc.vector.tensor_reduce(
                            rowstage[:, k:k + 1], s2[:],
                            axis=mybir.AxisListType.X, op=mybir.AluOpType.max,
                        )
                        cs = slice(c * _CHUNK, (c + 1) * _CHUNK)
                        nc.vector.tensor_tensor(
                            out=colacc[:, cs], in0=scr[:], in1=colacc[:, cs],
                            op=mybir.AluOpType.max,
                        )
                    # chunk done: 128-way partition max on GPSIMD (overlaps
                    # the next chunk's matmuls/DVE work)
                    cs = slice(c * _CHUNK, (c + 1) * _CHUNK)
                    nc.gpsimd.partition_all_reduce(
                        colred[:, cs], colacc[:, cs],
                        channels=_P, reduce_op=bass_isa.ReduceOp.max,
                    )
                    nc.sync.dma_start(out=colmax_d[:, cs], in_=colred[0:1, cs])

            # rowstage holds max(-d); combine chunks, host negates.
            rowmin_sb = io.tile([_P, r_tiles], F32)
            nc.vector.tensor_reduce(
                rowmin_sb[:],
                rowstage[:].rearrange("p (i c) -> p i c", c=n_chunks),
                axis=mybir.AxisListType.X, op=mybir.AluOpType.max,
            )
            nc.sync.dma_start(out=rowmin_d[:, :], in_=rowmin_sb[:])
    nc.finalize()
    return nc


def _get_nc(r_tiles: int, m_pad: int):
    key = (r_tiles, m_pad)
    if key not in _NC_CACHE:
        _NC_CACHE[key] = _build_nc(r_tiles, m_pad)
    return _NC_CACHE[key]


def _chamfer_numpy(p, t, mask):
    """Blocked numpy fallback (exact), for odd configurations."""
    B = p.shape[0]
    per_sample = np.zeros(B, dtype=np.float64)
    for b in range(B):
        pb, tb = p[b], t[b]
        tn = (tb * tb).sum(1)
        pn = (pb * pb).sum(1)
        rowmin = np.full(pb.shape[0], np.inf, dtype=np.float32)
        colmin = np.full(tb.shape[0], np.float32(BIG), dtype=np.float32)
        step = 512
        for i in range(0, pb.shape[0], step):
            d = (pn[i:i + step, None] + tn[None, :]
                 - 2.0 * (pb[i:i + step] @ tb.T)).astype(np.float32)
            d = np.maximum(d, 0.0)
            rowmin[i:i + step] = d.min(axis=1)
            mrows = mask[b, i:i + step]
            if mrows.any():
                colmin = np.minimum(colmin, d[mrows].min(axis=0))
        cnt = max(int(mask[b].sum()), 1)
        m1 = np.sqrt(rowmin[mask[b]]).sum() / cnt
        m2 = np.sqrt(colmin).mean()
        per_sample[b] = 0.5 * (m1 + m2)
    return np.asarray(per_sample.mean(), dtype=np.float32)


def kernel(pred_pc, target, label, nums, dense_nums):
    B = int(np.asarray(nums).shape[0])
    p = np.ascontiguousarray(np.asarray(pred_pc, dtype=np.float32)).reshape(B, -1, 3)
    t = np.ascontiguousarray(np.asarray(target, dtype=np.float32)).reshape(B, -1, 3)
    N = p.shape[1]
    M = t.shape[1]
    mask = (np.asarray(label).reshape(B, N) == 1)

    if B < 1 or B > 8 or M < 1:
        return _chamfer_numpy(p, t, mask)

    cps = max(1, 8 // B)          # cores per sample
    n_cores = B * cps
    m_pad = ((M + _CHUNK - 1) // _CHUNK) * _CHUNK

    # Split each sample's valid pred points across its cores.
    parts = []                    # (sample, pts[r,3]) per core
    for b in range(B):
        pv = p[b][mask[b]]
        for chunk in np.array_split(pv, cps, axis=0):
            parts.append((b, np.ascontiguousarray(chunk)))
    rmax = max(c.shape[0] for _, c in parts)
    # Rows past a full 128-tile boundary would cost a whole extra matmul
    # pass; when that overflow is small, handle those rows on the host.
    r_floor = max(_P, (rmax // _P) * _P)
    if 0 < rmax - r_floor <= 48:
        R = r_floor
    else:
        R = max(_P, ((rmax + _P - 1) // _P) * _P)
    r_tiles = R // _P

    nc = _get_nc(r_tiles, m_pad)

    in_maps = []
    for b, pts in parts:
        r = min(pts.shape[0], R)
        inp = np.zeros((5, R + m_pad), dtype=np.float32)
        if r > 0:
            inp[0:3, :r] = -2.0 * pts[:r].T
            inp[4, :r] = (pts[:r] * pts[:r]).sum(1)
        inp[3, :R] = 1.0
        inp[4, r:R] = BIG
        inp[0:3, R:R + M] = t[b].T
        inp[3, R:R + M] = (t[b] * t[b]).sum(1)
        if m_pad > M:               # padding cols must never win a row-min
            inp[3, R + M:] = BIG
        inp[4, R:] = 1.0
        in_maps.append({"inp": inp})

    res = run_bass_kernel_spmd(nc, in_maps, core_ids=list(range(n_cores)))

    per_sample = np.zeros(B, dtype=np.float64)
    for b in range(B):
        d1_sum = 0.0
        colmin = np.full(M, np.float32(BIG), dtype=np.float32)
        tn_b = None
        for h in range(cps):
            core = b * cps + h
            pts = parts[core][1]
            r = min(pts.shape[0], R)
            out = res.results[core]
            if r > 0:
                rowmin = -out["rowmin"].T.ravel()[:r]      # n = i*128 + p
                d1_sum += np.sqrt(np.maximum(rowmin, 0.0)).sum(dtype=np.float64)
            colmin = np.minimum(colmin, -out["colmax"][0, :M])
            if pts.shape[0] > R:                           # host overflow rows
                hp = pts[R:]
                if tn_b is None:
                    tn_b = (t[b] * t[b]).sum(1)
                d = ((hp * hp).sum(1)[:, None] + tn_b[None, :]
                     - 2.0 * (hp @ t[b].T)).astype(np.float32)
                d = np.maximum(d, 0.0)
                d1_sum += np.sqrt(d.min(axis=1)).sum(dtype=np.float64)
                colmin = np.minimum(colmin, d.min(axis=0))
        nv = int(mask[b].sum())
        cnt = max(nv, 1)
        m1 = d1_sum / cnt
        if nv == 0:
            colmin[:] = BIG        # reference: all rows masked -> d = BIG
        m2 = np.sqrt(np.maximum(colmin, 0.0)).mean(dtype=np.float64)
        per_sample[b] = 0.5 * (m1 + m2)

    return np.asarray(per_sample.mean(), dtype=np.float32)


# revision 25
# speedup vs baseline: 1.0267x; 1.0267x over previous
"""Chamfer loss (ChamferDistanceL1-style) Trainium2 Bass kernel.

Problem: B=4 samples, N=M=4096 points, 3D. loss = mean_b 0.5*(m1_b + m2_b)
  m1 = masked mean over valid pred points of sqrt(min_m d[n,m])
  m2 = mean over target points of sqrt(min over *valid* n of d[n,m])
  d[n,m] = max(|p_n|^2 + |t_m|^2 - 2 p.t, 0)

Strategy (8 NeuronCores):
  - Host compacts each sample's pred points to the valid (label==1) subset
    (~halves the work), splits them across 2 cores -> 8 cores = 4 samples x 2.
  - Distances are produced by a single K=5 fp32 matmul per tile:
      lhsT col n = [-2px, -2py, -2pz, 1, |p_n|^2 (+BIG if padding)]
      rhs  col m = [ tx,   ty,   tz,  |t_m|^2, 1]
    so PSUM holds d[n,m] directly (before the max(.,0) clamp).
  - Per PSUM chunk [128, 2048]:
      ACT: negated fp16 copy PSUM -> SBUF (sole PSUM consumer, so the fp32
           matmuls never stall on PSUM slots)
      DVE: row max(-d) via two fp16 TT-max tree levels (2x DVE mode) plus a
           final 1x tensor_reduce; fp16 TT-max into the negated column
           accumulator (also 2x mode)
  - GPSIMD finishes each chunk's 128-way partition max (overlapped with the
    next chunk's compute); host does the final clamp/sqrt/means (tiny).
  - fp16 is a value rounding of already-exact fp32 distances (max-combining
    is exact in fp16): measured loss error ~1e-6 relative.
"""

import numpy as np

import concourse.bacc as bacc
import concourse.bass_isa as bass_isa
import concourse.tile as tile
from concourse import mybir
from concourse.bass_utils import run_bass_kernel_spmd

F32 = mybir.dt.float32
F16 = mybir.dt.float16
BIG = np.float32(1e10)  # matches the reference's masking constant
_NC_CACHE = {}

_P = 128          # partitions / rows per weight tile
_MM_FREE = 512    # fp32 matmul moving-dim limit (one PSUM bank)
_CHUNK = 2048     # PSUM chunk (4 banks); 2 bufs = all 8 banks


def _chunk_widths(m_pad: int):
    """Column-chunk widths. The final 2048 is split (1536, 512) so the last
    chunk's serial GPSIMD partition-reduce tail is 4x shorter; earlier
    chunks' reduces hide under subsequent compute."""
    assert m_pad % _CHUNK == 0
    return [_CHUNK] * (m_pad // _CHUNK - 1) + [1536, 512]


def _build_nc(r_tiles: int, m_pad: int):
    """Build + finalize the per-core Bass program for R=128*r_tiles pred rows
    and m_pad (multiple of _CHUNK) target columns."""
    R = r_tiles * _P
    widths = _chunk_widths(m_pad)
    n_chunks = len(widths)

    nc = bacc.Bacc("TRN2", target_bir_lowering=False)
    inp = nc.dram_tensor("inp", [5, R + m_pad], F32, kind="ExternalInput")
    rowmin_d = nc.dram_tensor("rowmin", [_P, r_tiles], F32, kind="ExternalOutput")
    colmax_d = nc.dram_tensor("colmax", [1, m_pad], F32, kind="ExternalOutput")
    warm_d = nc.dram_tensor("warm", [_P, 1], F32, kind="ExternalOutput")

    with tile.TileContext(nc) as tc:
        with tc.tile_pool(name="io", bufs=1) as io, \
             tc.tile_pool(name="ps", bufs=2, space="PSUM") as psp:
            # PE warmup: a dummy matmul during the input DMA starts the HAM
            # clock-gate ramp so real matmuls run closer to full clock.
            wsrc = io.tile([5, _MM_FREE], F32)
            nc.vector.memset(wsrc[:], 0.0)
            wps = psp.tile([_P, _MM_FREE], F32, tag="ps")
            nc.tensor.matmul(wps[:], wsrc[:, 0:_P], wsrc[:],
                             start=True, stop=True)
            warm_sb = io.tile([_P, 1], F32)
            nc.vector.tensor_reduce(warm_sb[:], wps[:],
                                    axis=mybir.AxisListType.X,
                                    op=mybir.AluOpType.max)
            nc.sync.dma_start(out=warm_d[:, :], in_=warm_sb[:])

            # weights first (small), then one DMA per rhs chunk: the first
            # matmuls only wait on their own chunk's DMA.
            in_sb = io.tile([5, R + m_pad], F32)
            nc.sync.dma_start(out=in_sb[:, :R], in_=inp[:, :R])
            off = 0
            for w in widths:
                cs = slice(R + off, R + off + w)
                nc.sync.dma_start(out=in_sb[:, cs], in_=inp[:, cs])
                off += w

            # negated fp16 column accumulator: holds max(-d) = -min(d)
            colacc = io.tile([_P, m_pad], F16)
            nc.any.memset(colacc[:], -60000.0)
            colred = io.tile([_P, m_pad], F32)

            rowstage = io.tile([_P, r_tiles * n_chunks], F32)

            with tc.tile_pool(name="scr", bufs=3) as scrp:
                off = 0
                for c, w in enumerate(widths):
                    for i in range(r_tiles):
                        lhsT = in_sb[:, i * _P:(i + 1) * _P]
                        ps = psp.tile([_P, w], F32, tag="ps")
                        for s in range(w // _MM_FREE):
                            col0 = R + off + s * _MM_FREE
                            nc.tensor.matmul(
                                ps[:, s * _MM_FREE:(s + 1) * _MM_FREE],
                                lhsT,
                                in_sb[:, col0:col0 + _MM_FREE],
                                start=True, stop=True,
                            )
                        # ACT: scr = -d in fp16; frees the PSUM slot fast so
                        # the PE never stalls. Both reductions read scr.
                        scr = scrp.tile([_P, w], F16, tag="scr")
                        nc.scalar.mul(scr[:], ps[:], -1.0)
                        # row max(-d): two fp16 TT-max tree levels run in the
                        # DVE 2x mode before the (1x-only) tensor_reduce.
                        h1 = w // 2
                        s1 = scrp.tile([_P, h1], F16, tag="s1")
                        nc.vector.tensor_tensor(out=s1[:], in0=scr[:, :h1],
                                                in1=scr[:, h1:],
                                                op=mybir.AluOpType.max)
                        h2 = h1 // 2
                        s2 = scrp.tile([_P, h2], F16, tag="s2")
                        nc.vector.tensor_tensor(out=s2[:], in0=s1[:, :h2],
                                                in1=s1[:, h2:],
                                                op=mybir.AluOpType.max)
                        k = i * n_chunks + c
                        nc.vector.tensor_reduce(
                            rowstage[:, k:k + 1], s2[:],
                            axis=mybir.AxisListType.X, op=mybir.AluOpType.max,
                        )
                        cs = slice(off, off + w)
                        nc.vector.tensor_tensor(
                            out=colacc[:, cs], in0=scr[:], in1=colacc[:, cs],
                            op=mybir.AluOpType.max,
                        )
                    # chunk done: 128-way partition max on GPSIMD (overlaps
                    # the next chunk's matmuls/DVE work)
                    cs = slice(off, off + w)
                    nc.gpsimd.partition_all_reduce(
                        colred[:, cs], colacc[:, cs],
                        channels=_P, reduce_op=bass_isa.ReduceOp.max,
                    )
                    nc.sync.dma_start(out=colmax_d[:, cs], in_=colred[0:1, cs])
                    off += w

            # rowstage holds max(-d); combine chunks, host negates.
            rowmin_sb = io.tile([_P, r_tiles], F32)
            nc.vector.tensor_reduce(
                rowmin_sb[:],
                rowstage[:].rearrange("p (i c) -> p i c", c=n_chunks),
                axis=mybir.AxisListType.X, op=mybir.AluOpType.max,
            )
            nc.sync.dma_start(out=rowmin_d[:, :], in_=rowmin_sb[:])
    nc.finalize()
    return nc


def _get_nc(r_tiles: int, m_pad: int):
    key = (r_tiles, m_pad)
    if key not in _NC_CACHE:
        _NC_CACHE[key] = _build_nc(r_tiles, m_pad)
    return _NC_CACHE[key]


def _chamfer_numpy(p, t, mask):
    """Blocked numpy fallback (exact), for odd configurations."""
    B = p.shape[0]
    per_sample = np.zeros(B, dtype=np.float64)
    for b in range(B):
        pb, tb = p[b], t[b]
        tn = (tb * tb).sum(1)
        pn = (pb * pb).sum(1)
        rowmin = np.full(pb.shape[0], np.inf, dtype=np.float32)
        colmin = np.full(tb.shape[0], np.float32(BIG), dtype=np.float32)
        step = 512
        for i in range(0, pb.shape[0], step):
            d = (pn[i:i + step, None] + tn[None, :]
                 - 2.0 * (pb[i:i + step] @ tb.T)).astype(np.float32)
            d = np.maximum(d, 0.0)
            rowmin[i:i + step] = d.min(axis=1)
            mrows = mask[b, i:i + step]
            if mrows.any():
                colmin = np.minimum(colmin, d[mrows].min(axis=0))
        cnt = max(int(mask[b].sum()), 1)
        m1 = np.sqrt(rowmin[mask[b]]).sum() / cnt
        m2 = np.sqrt(colmin).mean()
        per_sample[b] = 0.5 * (m1 + m2)
    return np.asarray(per_sample.mean(), dtype=np.float32)


def kernel(pred_pc, target, label, nums, dense_nums):
    B = int(np.asarray(nums).shape[0])
    p = np.ascontiguousarray(np.asarray(pred_pc, dtype=np.float32)).reshape(B, -1, 3)
    t = np.ascontiguousarray(np.asarray(target, dtype=np.float32)).reshape(B, -1, 3)
    N = p.shape[1]
    M = t.shape[1]
    mask = (np.asarray(label).reshape(B, N) == 1)

    if B < 1 or B > 8 or M < 1:
        return _chamfer_numpy(p, t, mask)

    cps = max(1, 8 // B)          # cores per sample
    n_cores = B * cps
    m_pad = ((M + _CHUNK - 1) // _CHUNK) * _CHUNK

    # Split each sample's valid pred points across its cores.
    parts = []                    # (sample, pts[r,3]) per core
    for b in range(B):
        pv = p[b][mask[b]]
        for chunk in np.array_split(pv, cps, axis=0):
            parts.append((b, np.ascontiguousarray(chunk)))
    rmax = max(c.shape[0] for _, c in parts)
    # Rows past a full 128-tile boundary would cost a whole extra matmul
    # pass; when that overflow is small, handle those rows on the host.
    r_floor = max(_P, (rmax // _P) * _P)
    if 0 < rmax - r_floor <= 48:
        R = r_floor
    else:
        R = max(_P, ((rmax + _P - 1) // _P) * _P)
    r_tiles = R // _P

    nc = _get_nc(r_tiles, m_pad)

    in_maps = []
    for b, pts in parts:
        r = min(pts.shape[0], R)
        inp = np.zeros((5, R + m_pad), dtype=np.float32)
        if r > 0:
            inp[0:3, :r] = -2.0 * pts[:r].T
            inp[4, :r] = (pts[:r] * pts[:r]).sum(1)
        inp[3, :R] = 1.0
        inp[4, r:R] = BIG
        inp[0:3, R:R + M] = t[b].T
        inp[3, R:R + M] = (t[b] * t[b]).sum(1)
        if m_pad > M:               # padding cols must never win a row-min
            inp[3, R + M:] = BIG
        inp[4, R:] = 1.0
        in_maps.append({"inp": inp})

    res = run_bass_kernel_spmd(nc, in_maps, core_ids=list(range(n_cores)))

    per_sample = np.zeros(B, dtype=np.float64)
    for b in range(B):
        d1_sum = 0.0
        colmin = np.full(M, np.float32(BIG), dtype=np.float32)
        tn_b = None
        for h in range(cps):
            core = b * cps + h
            pts = parts[core][1]
            r = min(pts.shape[0], R)
            out = res.results[core]
            if r > 0:
                rowmin = -out["rowmin"].T.ravel()[:r]      # n = i*128 + p
                d1_sum += np.sqrt(np.maximum(rowmin, 0.0)).sum(dtype=np.float64)
            colmin = np.minimum(colmin, -out["colmax"][0, :M])
            if pts.shape[0] > R:                           # host overflow rows
                hp = pts[R:]
                if tn_b is None:
                    tn_b = (t[b] * t[b]).sum(1)
                d = ((hp * hp).sum(1)[:, None] + tn_b[None, :]
                     - 2.0 * (hp @ t[b].T)).astype(np.float32)
                d = np.maximum(d, 0.0)
                d1_sum += np.sqrt(d.min(axis=1)).sum(dtype=np.float64)
                colmin = np.minimum(colmin, d.min(axis=0))
        nv = int(mask[b].sum())
        cnt = max(nv, 1)
        m1 = d1_sum / cnt
        if nv == 0:
            colmin[:] = BIG        # reference: all rows masked -> d = BIG
        m2 = np.sqrt(np.maximum(colmin, 0.0)).mean(dtype=np.float64)
        per_sample[b] = 0.5 * (m1 + m2)

    return np.asarray(per_sample.mean(), dtype=np.float32)


# revision 26
# speedup vs baseline: 1.0400x; 1.0129x over previous
"""Chamfer loss (ChamferDistanceL1-style) Trainium2 Bass kernel.

Problem: B=4 samples, N=M=4096 points, 3D. loss = mean_b 0.5*(m1_b + m2_b)
  m1 = masked mean over valid pred points of sqrt(min_m d[n,m])
  m2 = mean over target points of sqrt(min over *valid* n of d[n,m])
  d[n,m] = max(|p_n|^2 + |t_m|^2 - 2 p.t, 0)

Strategy (8 NeuronCores):
  - Host compacts each sample's pred points to the valid (label==1) subset
    (~halves the work), splits them across 2 cores -> 8 cores = 4 samples x 2.
  - Distances are produced by a single K=5 fp32 matmul per tile:
      lhsT col n = [-2px, -2py, -2pz, 1, |p_n|^2 (+BIG if padding)]
      rhs  col m = [ tx,   ty,   tz,  |t_m|^2, 1]
    so PSUM holds d[n,m] directly (before the max(.,0) clamp).
  - Per PSUM chunk [128, 2048]:
      ACT: negated fp16 copy PSUM -> SBUF (sole PSUM consumer, so the fp32
           matmuls never stall on PSUM slots)
      DVE: row max(-d) via two fp16 TT-max tree levels (2x DVE mode) plus a
           final 1x tensor_reduce; fp16 TT-max into the negated column
           accumulator (also 2x mode)
  - GPSIMD finishes each chunk's 128-way partition max (overlapped with the
    next chunk's compute); host does the final clamp/sqrt/means (tiny).
  - fp16 is a value rounding of already-exact fp32 distances (max-combining
    is exact in fp16): measured loss error ~1e-6 relative.
"""

import numpy as np

import concourse.bacc as bacc
import concourse.bass_isa as bass_isa
import concourse.tile as tile
from concourse import mybir
from concourse.bass_utils import run_bass_kernel_spmd

F32 = mybir.dt.float32
F16 = mybir.dt.float16
BIG = np.float32(1e10)  # matches the reference's masking constant
_NC_CACHE = {}

_P = 128          # partitions / rows per weight tile
_MM_FREE = 512    # fp32 matmul moving-dim limit (one PSUM bank)
_CHUNK = 2048     # PSUM chunk (4 banks); 2 bufs = all 8 banks


def _chunk_widths(m_pad: int):
    """Column-chunk widths (each a multiple of 512, max 2048 = 4 PSUM banks).
    A small first chunk shortens the DMA lead-in before the first matmul; a
    small last chunk shrinks the serial GPSIMD partition-reduce tail 4x;
    earlier chunks' reduces hide under subsequent compute. The (1024, 1536,
    1024, 512) split measured best in the cost-model sweep for m_pad=4096."""
    assert m_pad % _CHUNK == 0
    if m_pad == 4096:
        return [1024, 1536, 1024, 512]
    return [_CHUNK] * (m_pad // _CHUNK - 1) + [1536, 512]


def _build_nc(r_tiles: int, m_pad: int):
    """Build + finalize the per-core Bass program for R=128*r_tiles pred rows
    and m_pad (multiple of _CHUNK) target columns."""
    R = r_tiles * _P
    widths = _chunk_widths(m_pad)
    n_chunks = len(widths)

    nc = bacc.Bacc("TRN2", target_bir_lowering=False)
    inp = nc.dram_tensor("inp", [5, R + m_pad], F32, kind="ExternalInput")
    rowmin_d = nc.dram_tensor("rowmin", [_P, r_tiles], F32, kind="ExternalOutput")
    colmax_d = nc.dram_tensor("colmax", [1, m_pad], F32, kind="ExternalOutput")
    warm_d = nc.dram_tensor("warm", [_P, 1], F32, kind="ExternalOutput")

    with tile.TileContext(nc) as tc:
        with tc.tile_pool(name="io", bufs=1) as io, \
             tc.tile_pool(name="ps", bufs=2, space="PSUM") as psp:
            # PE warmup: a dummy matmul during the input DMA starts the HAM
            # clock-gate ramp so real matmuls run closer to full clock.
            wsrc = io.tile([5, _MM_FREE], F32)
            nc.vector.memset(wsrc[:], 0.0)
            wps = psp.tile([_P, _MM_FREE], F32, tag="ps")
            nc.tensor.matmul(wps[:], wsrc[:, 0:_P], wsrc[:],
                             start=True, stop=True)
            warm_sb = io.tile([_P, 1], F32)
            nc.vector.tensor_reduce(warm_sb[:], wps[:],
                                    axis=mybir.AxisListType.X,
                                    op=mybir.AluOpType.max)
            nc.sync.dma_start(out=warm_d[:, :], in_=warm_sb[:])

            # weights first (small), then one DMA per rhs chunk: the first
            # matmuls only wait on their own chunk's DMA.
            in_sb = io.tile([5, R + m_pad], F32)
            nc.sync.dma_start(out=in_sb[:, :R], in_=inp[:, :R])
            off = 0
            for w in widths:
                cs = slice(R + off, R + off + w)
                nc.sync.dma_start(out=in_sb[:, cs], in_=inp[:, cs])
                off += w

            # negated fp16 column accumulator: holds max(-d) = -min(d)
            colacc = io.tile([_P, m_pad], F16)
            nc.any.memset(colacc[:], -60000.0)
            colred = io.tile([_P, m_pad], F32)

            rowstage = io.tile([_P, r_tiles * n_chunks], F32)

            with tc.tile_pool(name="scr", bufs=3) as scrp:
                off = 0
                for c, w in enumerate(widths):
                    for i in range(r_tiles):
                        lhsT = in_sb[:, i * _P:(i + 1) * _P]
                        ps = psp.tile([_P, w], F32, tag="ps")
                        for s in range(w // _MM_FREE):
                            col0 = R + off + s * _MM_FREE
                            nc.tensor.matmul(
                                ps[:, s * _MM_FREE:(s + 1) * _MM_FREE],
                                lhsT,
                                in_sb[:, col0:col0 + _MM_FREE],
                                start=True, stop=True,
                            )
                        # ACT: scr = -d in fp16; frees the PSUM slot fast so
                        # the PE never stalls. Both reductions read scr.
                        scr = scrp.tile([_P, w], F16, tag="scr")
                        nc.scalar.mul(scr[:], ps[:], -1.0)
                        # row max(-d): two fp16 TT-max tree levels run in the
                        # DVE 2x mode before the (1x-only) tensor_reduce.
                        h1 = w // 2
                        s1 = scrp.tile([_P, h1], F16, tag="s1")
                        nc.vector.tensor_tensor(out=s1[:], in0=scr[:, :h1],
                                                in1=scr[:, h1:],
                                                op=mybir.AluOpType.max)
                        h2 = h1 // 2
                        s2 = scrp.tile([_P, h2], F16, tag="s2")
                        nc.vector.tensor_tensor(out=s2[:], in0=s1[:, :h2],
                                                in1=s1[:, h2:],
                                                op=mybir.AluOpType.max)
                        k = i * n_chunks + c
                        nc.vector.tensor_reduce(
                            rowstage[:, k:k + 1], s2[:],
                            axis=mybir.AxisListType.X, op=mybir.AluOpType.max,
                        )
                        cs = slice(off, off + w)
                        nc.vector.tensor_tensor(
                            out=colacc[:, cs], in0=scr[:], in1=colacc[:, cs],
                            op=mybir.AluOpType.max,
                        )
                    # chunk done: 128-way partition max on GPSIMD (overlaps
                    # the next chunk's matmuls/DVE work)
                    cs = slice(off, off + w)
                    nc.gpsimd.partition_all_reduce(
                        colred[:, cs], colacc[:, cs],
                        channels=_P, reduce_op=bass_isa.ReduceOp.max,
                    )
                    nc.sync.dma_start(out=colmax_d[:, cs], in_=colred[0:1, cs])
                    off += w

            # rowstage holds max(-d); combine chunks, host negates.
            rowmin_sb = io.tile([_P, r_tiles], F32)
            nc.vector.tensor_reduce(
                rowmin_sb[:],
                rowstage[:].rearrange("p (i c) -> p i c", c=n_chunks),
                axis=mybir.AxisListType.X, op=mybir.AluOpType.max,
            )
            nc.sync.dma_start(out=rowmin_d[:, :], in_=rowmin_sb[:])
    nc.finalize()
    return nc


def _get_nc(r_tiles: int, m_pad: int):
    key = (r_tiles, m_pad)
    if key not in _NC_CACHE:
        _NC_CACHE[key] = _build_nc(r_tiles, m_pad)
    return _NC_CACHE[key]


def _chamfer_numpy(p, t, mask):
    """Blocked numpy fallback (exact), for odd configurations."""
    B = p.shape[0]
    per_sample = np.zeros(B, dtype=np.float64)
    for b in range(B):
        pb, tb = p[b], t[b]
        tn = (tb * tb).sum(1)
        pn = (pb * pb).sum(1)
        rowmin = np.full(pb.shape[0], np.inf, dtype=np.float32)
        colmin = np.full(tb.shape[0], np.float32(BIG), dtype=np.float32)
        step = 512
        for i in range(0, pb.shape[0], step):
            d = (pn[i:i + step, None] + tn[None, :]
                 - 2.0 * (pb[i:i + step] @ tb.T)).astype(np.float32)
            d = np.maximum(d, 0.0)
            rowmin[i:i + step] = d.min(axis=1)
            mrows = mask[b, i:i + step]
            if mrows.any():
                colmin = np.minimum(colmin, d[mrows].min(axis=0))
        cnt = max(int(mask[b].sum()), 1)
        m1 = np.sqrt(rowmin[mask[b]]).sum() / cnt
        m2 = np.sqrt(colmin).mean()
        per_sample[b] = 0.5 * (m1 + m2)
    return np.asarray(per_sample.mean(), dtype=np.float32)


def kernel(pred_pc, target, label, nums, dense_nums):
    B = int(np.asarray(nums).shape[0])
    p = np.ascontiguousarray(np.asarray(pred_pc, dtype=np.float32)).reshape(B, -1, 3)
    t = np.ascontiguousarray(np.asarray(target, dtype=np.float32)).reshape(B, -1, 3)
    N = p.shape[1]
    M = t.shape[1]
    mask = (np.asarray(label).reshape(B, N) == 1)

    if B < 1 or B > 8 or M < 1:
        return _chamfer_numpy(p, t, mask)

    cps = max(1, 8 // B)          # cores per sample
    n_cores = B * cps
    m_pad = ((M + _CHUNK - 1) // _CHUNK) * _CHUNK

    # Split each sample's valid pred points across its cores.
    parts = []                    # (sample, pts[r,3]) per core
    for b in range(B):
        pv = p[b][mask[b]]
        for chunk in np.array_split(pv, cps, axis=0):
            parts.append((b, np.ascontiguousarray(chunk)))
    rmax = max(c.shape[0] for _, c in parts)
    # Rows past a full 128-tile boundary would cost a whole extra matmul
    # pass; when that overflow is small, handle those rows on the host.
    r_floor = max(_P, (rmax // _P) * _P)
    if 0 < rmax - r_floor <= 48:
        R = r_floor
    else:
        R = max(_P, ((rmax + _P - 1) // _P) * _P)
    r_tiles = R // _P

    nc = _get_nc(r_tiles, m_pad)

    in_maps = []
    for b, pts in parts:
        r = min(pts.shape[0], R)
        inp = np.zeros((5, R + m_pad), dtype=np.float32)
        if r > 0:
            inp[0:3, :r] = -2.0 * pts[:r].T
            inp[4, :r] = (pts[:r] * pts[:r]).sum(1)
        inp[3, :R] = 1.0
        inp[4, r:R] = BIG
        inp[0:3, R:R + M] = t[b].T
        inp[3, R:R + M] = (t[b] * t[b]).sum(1)
        if m_pad > M:               # padding cols must never win a row-min
            inp[3, R + M:] = BIG
        inp[4, R:] = 1.0
        in_maps.append({"inp": inp})

    res = run_bass_kernel_spmd(nc, in_maps, core_ids=list(range(n_cores)))

    per_sample = np.zeros(B, dtype=np.float64)
    for b in range(B):
        d1_sum = 0.0
        colmin = np.full(M, np.float32(BIG), dtype=np.float32)
        tn_b = None
        for h in range(cps):
            core = b * cps + h
            pts = parts[core][1]
            r = min(pts.shape[0], R)
            out = res.results[core]
            if r > 0:
                rowmin = -out["rowmin"].T.ravel()[:r]      # n = i*128 + p
                d1_sum += np.sqrt(np.maximum(rowmin, 0.0)).sum(dtype=np.float64)
            colmin = np.minimum(colmin, -out["colmax"][0, :M])
            if pts.shape[0] > R:                           # host overflow rows
                hp = pts[R:]
                if tn_b is None:
                    tn_b = (t[b] * t[b]).sum(1)
                d = ((hp * hp).sum(1)[:, None] + tn_b[None, :]
                     - 2.0 * (hp @ t[b].T)).astype(np.float32)
                d = np.maximum(d, 0.0)
                d1_sum += np.sqrt(d.min(axis=1)).sum(dtype=np.float64)
                colmin = np.minimum(colmin, d.min(axis=0))
        nv = int(mask[b].sum())
        cnt = max(nv, 1)
        m1 = d1_sum / cnt
        if nv == 0:
            colmin[:] = BIG        # reference: all rows masked -> d = BIG
        m2 = np.sqrt(np.maximum(colmin, 0.0)).mean(dtype=np.float64)
        per_sample[b] = 0.5 * (m1 + m2)

    return np.asarray(per_sample.mean(), dtype=np.float32)
